# revision 1
# baseline (speedup 1.0000x reference)
"""DialogueGCN forward on 8 Trainium2 NeuronCores (Bass/Tile).

kernel(**inputs) -> np.ndarray [8192, 6] log-probs, matching reference().

Sharding: nodes row-sharded 1024/core; each core owns edges into its dst
strip. No device gathers: stage-1 edge features are host-permuted x rows
(xg, fp16) streamed contiguously and segment-summed into (dst,rel)
buckets via narrow selection matmuls, then transformed with W (linearity
of RGCN). Stage-2 reads the AllGathered h1 contiguously and aggregates
with a dense count matrix streamed as the moving matmul operand.

Precision: the attention logits span +-600, so absolute logit error must
stay ~1e-2 — everything feeding S (W, h1, h2, before, beta_w/gc/root
weights) is kept at f32/f32r; h1 crosses cores as an fp16 hi+lo pair
(matmul dtype pairing), h2 crosses in f32. Keys x-part, V, P, em2 and
the head run in fp16. Softmax uses a true per-row max (S rows in SBUF).
"""
import numpy as np

import concourse.bass as bass
import concourse.tile as tile
import concourse.mybir as mybir
from concourse import bacc
from concourse.bass_utils import run_bass_kernel_spmd

f32 = mybir.dt.float32
f32r = mybir.dt.float32r
f16 = mybir.dt.float16

N, E, F, H, R, NB, NC = 8192, 680000, 200, 100, 8, 30, 6
CORES = 8
NPC = N // CORES            # 1024 dst rows per core
NG = NPC // 32              # 32-dst groups per core (32)
NBIN = NG * R               # bins per core (256)
KB = N // 128               # key blocks (64)

AF = mybir.ActivationFunctionType
ALU = mybir.AluOpType
AX = mybir.AxisListType

_ker_cache = {}
_last_res = None


# ------------------------------------------------------------------ host prep
def _prep(x, edge_index, edge_type):
    src = np.asarray(edge_index[0], np.int64)
    dst = np.asarray(edge_index[1], np.int64)
    et = np.asarray(edge_type, np.int64)

    deg = np.bincount(dst * R + et, minlength=N * R).astype(np.float64)
    inv = np.where(deg > 0, 1.0 / np.maximum(deg, 1.0), 0.0).astype(np.float32)
    invv = inv[dst * R + et]                       # per-edge weight

    core = dst >> 10
    g = (dst & 1023) >> 5
    binid = ((core * NG + g) << 3) | et            # [0, CORES*NBIN)
    order = np.argsort(binid, kind="stable")
    cnt = np.bincount(binid, minlength=CORES * NBIN).reshape(CORES, NBIN)
    B1 = (-(-cnt // 128)).max(axis=0)              # blocks per bin (static)
    totB1 = int(B1.sum())
    bstart = np.concatenate([[0], np.cumsum(B1)])  # block offset per bin

    flat_cnt = cnt.reshape(-1)
    starts = np.concatenate([[0], np.cumsum(flat_cnt)])
    pos = np.arange(E, dtype=np.int64) - np.repeat(starts[:-1], flat_cnt)
    sbin = binid[order]
    blk_all = bstart[sbin % NBIN] + (pos >> 7)
    row_all = pos & 127

    x16 = np.asarray(x, np.float32).astype(np.float16)
    per_core = []
    for c in range(CORES):
        sl = slice(starts[c * NBIN], starts[(c + 1) * NBIN])
        e = order[sl]
        bl = blk_all[sl]
        rw = row_all[sl]
        xg = np.zeros((128, totB1, F), np.float16)
        xg[rw, bl] = x16[src[e]]
        sel1 = np.zeros((128, totB1, 32), np.float16)
        sel1[rw, bl, dst[e] & 31] = invv[e]
        cnt2 = np.bincount(src[e] * NPC + (dst[e] & 1023), minlength=N * NPC)
        sel2 = np.ascontiguousarray(
            cnt2.reshape(KB, 128, NPC).transpose(1, 0, 2)).astype(np.float16)
        per_core.append(dict(
            xg=xg.reshape(128, totB1 * F),
            sel1=sel1.reshape(128, totB1 * 32),
            sel2=sel2.reshape(128, KB * NPC)))

    meta = dict(B1=[int(b) for b in B1], totB1=totB1)
    return per_core, meta


# ------------------------------------------------------------------ program
def _build(meta, phase="full"):
    B1 = meta["B1"]
    totB1 = meta["totB1"]

    nc = bacc.Bacc("TRN2", target_bir_lowering=False, debug=False,
                   num_devices=CORES)
    P = lambda n, s, d: nc.declare_dram_parameter(n, s, d, isOutput=False)

    xT16d = P("xT16", [F, N], f16)             # keys x-part
    xvd = P("xv", [128, KB * F], f16)          # emoV x-part, [p, kb, f]
    xTs32d = P("xTs32", [100, 2 * NPC], f32)   # own strip xT, f32
    basis32d = P("basis32", [NB, F * H], f32)
    compT32d = P("compT32", [NB, R], f32)
    rootw32d = P("rootw32", [100, 2 * H], f32)
    rootb32d = P("rootb32", [1, H], f32)
    gcrel32d = P("gcrel32", [H, H], f32)
    gcrelb32d = P("gcrelb32", [1, H], f32)
    gcroot32d = P("gcroot32", [H, H], f32)
    betaw32d = P("betaw32", [100, 3 * 3 * H], f32)  # [f, fc, gc, h]
    betab32d = P("betab32", [1, 3 * H], f32)
    linwd = P("linw16", [100, 3 * H], f16)     # chunks [f, c, h]
    linbd = P("linb16", [1, H], f16)
    smaxwd = P("smaxw16", [H, NC], f16)
    smaxbd = P("smaxb16", [1, NC], f16)
    id16d = P("ident16", [128, 128], f16)
    id32d = P("ident32", [128, 128], f32)
    ones32d = P("ones32", [1, 512], f32)
    ones16d = P("ones16", [1, NPC], f16)
    xgd = P("xg", [128, totB1 * F], f16)
    sel1d = P("sel1", [128, totB1 * 32], f16)
    sel2d = P("sel2", [128, KB * NPC], f16)

    outd = nc.declare_dram_parameter("out", [NPC, NC], f32, isOutput=True)
    dbgd = None
    if phase != "full":
        dbgd = nc.declare_dram_parameter("dbg", [N, 2 * H], f16, isOutput=True)

    with tile.TileContext(nc, num_cores=CORES) as tc:
        with tc.tile_pool(name="dram", bufs=1, space="DRAM") as dram, \
             tc.tile_pool(name="persist", bufs=1) as pp:

            wtmp_d = dram.tile([R, F * H], f32, tag="wtmp")
            h1s_d = dram.tile([NPC, 2 * H], f16, tag="h1s")
            h1f_d = dram.tile([N, 2 * H], f16, tag="h1f")
            h2s_d = dram.tile([NPC, H], f32, tag="h2s")
            h2f_d = dram.tile([N, H], f32, tag="h2f")

            # --------- persistent SBUF (small) ---------
            id16 = pp.tile([128, 128], f16, tag="id16")
            nc.sync.dma_start(id16[:], id16d[:])
            id32 = pp.tile([128, 128], f32, tag="id32")
            nc.sync.dma_start(id32[:], id32d[:])
            ones16 = pp.tile([1, NPC], f16, tag="ones16")
            nc.sync.dma_start(ones16[:], ones16d[:])
            linw = pp.tile([100, 3, H], f16, tag="linw")
            nc.sync.dma_start(linw[:], linwd[:].rearrange(
                "p (c h) -> p c h", c=3))
            linb = pp.tile([1, H], f16, tag="linb")
            nc.sync.dma_start(linb[:], linbd[:])
            smaxw = pp.tile([H, NC], f16, tag="smaxw")
            nc.sync.dma_start(smaxw[:], smaxwd[:])
            smaxb = pp.tile([1, NC], f16, tag="smaxb")
            nc.sync.dma_start(smaxb[:], smaxbd[:])

            # f32 weights for the S-precision chain (freed before
            # attention: scoped in pool pm spanning phase W .. bef).
            # Plain fp32 matmuls (4 cyc/row) — f32r measured only ~13-bit.
            pm = tc.tile_pool(name="pm", bufs=1)
            pmp = pm.__enter__()
            onesr = pmp.tile([1, 512], f32, tag="onesr")
            nc.sync.dma_start(onesr[:], ones32d[:])
            rootwr = pmp.tile([100, 2, H], f32, tag="rootwr")
            nc.sync.dma_start(rootwr[:], rootw32d[:].rearrange(
                "p (c h) -> p c h", c=2))
            rootbr = pmp.tile([1, H], f32, tag="rootbr")
            nc.sync.dma_start(rootbr[:], rootb32d[:])
            gcrelr = pmp.tile([H, H], f32, tag="gcrelr")
            nc.sync.dma_start(gcrelr[:], gcrel32d[:])
            gcrelbr = pmp.tile([1, H], f32, tag="gcrelbr")
            nc.sync.dma_start(gcrelbr[:], gcrelb32d[:])
            gcrootr = pmp.tile([H, H], f32, tag="gcrootr")
            nc.sync.dma_start(gcrootr[:], gcroot32d[:])
            betawr = pmp.tile([100, 3, 3, H], f32, tag="betawr")
            nc.sync.dma_start(betawr[:], betaw32d[:].rearrange(
                "p (fc gc h) -> p fc gc h", fc=3, gc=3))
            betabr = pmp.tile([1, 3 * H], f32, tag="betabr")
            nc.sync.dma_start(betabr[:], betab32d[:])
            xTsr = pmp.tile([100, 2, NPC], f32, tag="xTsr")
            nc.sync.dma_start(xTsr[:], xTs32d[:].rearrange(
                "p (c n) -> p c n", c=2))
            Wtr = pmp.tile([100, 2, R, H], f32, tag="Wtr")
            h1Tr = pmp.tile([100, NPC], f32, tag="h1Tr")
            h2Tr = pmp.tile([100, NPC], f32, tag="h2Tr")

            # cross-phase tiles
            bef16x = pp.tile([100, 2, NPC], f16, tag="bef16x")
            bef2 = pp.tile([100, 2, NPC], f16, tag="bef2")  # h2-chunk hi/lo
            em2sb = pp.tile([128, 8, 300], f32, tag="em2sb")
            rsum = pp.tile([128, 8], f32, tag="rsum")

            # ================= phase W: relation weights (f32r) ==========
            with tc.tile_pool(name="pw", bufs=2) as pw, \
                 tc.tile_pool(name="pwc", bufs=3) as pwc, \
                 tc.tile_pool(name="psw", bufs=2, space="PSUM") as psw:
                compT = pw.tile([NB, R], f32, tag="compT")
                nc.sync.dma_start(compT[:], compT32d[:])
                CB = 2500
                for cb in range(F * H // CB):
                    bchunk = pw.tile([NB, CB], f32, tag="bchunk")
                    nc.sync.dma_start(
                        bchunk[:], basis32d[:, cb * CB:(cb + 1) * CB])
                    for t in range(CB // 500):
                        pwp = psw.tile([R, 500], f32, tag="pwp")
                        nc.tensor.matmul(pwp[:], compT[:],
                                         bchunk[:, t * 500:(t + 1) * 500],
                                         start=True, stop=True)
                        wsb = pwc.tile([R, 500], f32, tag="wsb")
                        nc.vector.tensor_copy(wsb[:], pwp[:])
                        nc.sync.dma_start(
                            wtmp_d[:, cb * CB + t * 500:
                                   cb * CB + (t + 1) * 500], wsb[:])
                # read back transposed: Wt[fp,fc,r,h] = W[r, fc*100+fp, h]
                for fc in range(2):
                    nc.sync.dma_start(
                        Wtr[:, fc, :, :],
                        wtmp_d[:, fc * 100 * H:(fc + 1) * 100 * H].rearrange(
                            "r (fp h) -> fp r h", h=H))

            # ================= stage 1: RGCN =================
            CH = 16                          # blocks per DMA chunk
            with tc.tile_pool(name="p1", bufs=3) as p1, \
                 tc.tile_pool(name="p1b", bufs=2) as p1b, \
                 tc.tile_pool(name="p1c", bufs=1) as p1c, \
                 tc.tile_pool(name="ps1", bufs=2, space="PSUM") as ps1, \
                 tc.tile_pool(name="ps1h", bufs=1, space="PSUM") as ps1h:
                aggsbr = p1c.tile([128, 2, R, NPC], f32, tag="aggsbr")
                h1T_ps = ps1h.tile([128, NPC], f32, tag="h1T_ps")

                # bin -> (group, rel); blocks laid out bin-major.
                # PSUM has_written is cleared bank-wide by start=True, so a
                # group's agg tile gets exactly ONE start (its first matmul)
                # and ONE stop (its last); fresh regions written with
                # start=False overwrite-and-set.
                binof = []
                for b_idx, nb in enumerate(B1):
                    binof += [b_idx] * nb
                gfirst = {}
                glast = {}
                for b in range(totB1):
                    gg = binof[b] >> 3
                    if gg not in gfirst:
                        gfirst[gg] = b
                    glast[gg] = b
                agg_ps = None
                gcur = -1
                bo = 0
                while bo < totB1:
                    k = min(CH, totB1 - bo)
                    xgt = p1.tile([128, CH, F], f16, tag="xgt")
                    nc.sync.dma_start(
                        xgt[:, 0:k, :],
                        xgd[:, bo * F:(bo + k) * F].rearrange(
                            "p (b f) -> p b f", f=F))
                    selt = p1.tile([128, CH, 32], f16, tag="selt")
                    nc.sync.dma_start(
                        selt[:, 0:k, :],
                        sel1d[:, bo * 32:(bo + k) * 32].rearrange(
                            "p (b d) -> p b d", d=32))
                    for j in range(k):
                        b = bo + j
                        bn = binof[b]
                        gg, rr = bn >> 3, bn & 7
                        if gg != gcur:
                            if agg_ps is not None:
                                nc.vector.tensor_copy(
                                    aggsbr[:, :, :, gcur * 32:(gcur + 1) * 32],
                                    agg_ps[:].rearrange(
                                        "p fc (r d) -> p fc r d", d=32))
                            agg_ps = ps1.tile([128, 2, R * 32], f32,
                                              tag="agg_ps")
                            gcur = gg
                        for fc in range(2):
                            nc.tensor.matmul(
                                agg_ps[0:100, fc, rr * 32:(rr + 1) * 32],
                                xgt[:, j, fc * 100:(fc + 1) * 100],
                                selt[:, j, :],
                                start=(b == gfirst[gg] and fc == 0),
                                stop=(b == glast[gg] and fc == 1))
                    bo += k
                nc.vector.tensor_copy(
                    aggsbr[:, :, :, gcur * 32:(gcur + 1) * 32],
                    agg_ps[:].rearrange("p fc (r d) -> p fc r d", d=32))

                # transform: h1T = sum_r W_r^T agg_r + root + bias (f32r)
                for hh in range(2):
                    hsl = slice(hh * 512, (hh + 1) * 512)
                    mmi = 0
                    for fc in range(2):
                        for rr in range(R):
                            nc.tensor.matmul(h1T_ps[0:100, hsl],
                                             Wtr[:, fc, rr, :],
                                             aggsbr[0:100, fc, rr, hsl],
                                             start=(mmi == 0), stop=False)
                            mmi += 1
                    for fc in range(2):
                        nc.tensor.matmul(h1T_ps[0:100, hsl], rootwr[:, fc, :],
                                         xTsr[:, fc, hsl],
                                         start=False, stop=False)
                    nc.tensor.matmul(h1T_ps[0:100, hsl], rootbr[:],
                                     onesr[:], start=False, stop=True)
                nc.vector.tensor_copy(h1Tr[:], h1T_ps[0:100, :])

                # h1 -> fp16 hi + lo pair, node-major strip, DRAM
                h1hi = p1b.tile([100, NPC], f16, tag="h1hi")
                nc.vector.tensor_copy(h1hi[:], h1T_ps[0:100, :])
                h1lo = p1b.tile([100, NPC], f16, tag="h1lo")
                nc.vector.tensor_tensor(h1lo[:], h1T_ps[0:100, :], h1hi[:],
                                        op=ALU.subtract)
                h1n = p1b.tile([128, 8, 2, H], f16, tag="h1n")
                for t in range(8):
                    for pr, part in enumerate((h1hi, h1lo)):
                        tp = ps1.tile([128, H], f16, tag="tp1")
                        nc.tensor.matmul(tp[:], part[:, t * 128:(t + 1) * 128],
                                         id16[0:100, 0:100], is_transpose=True,
                                         start=True, stop=True)
                        nc.vector.tensor_copy(h1n[:, t, pr, :], tp[:])
                nc.sync.dma_start(
                    h1s_d[:].rearrange("(b p) (pr h) -> p b pr h",
                                       p=128, pr=2), h1n[:])

            nc.gpsimd.collective_compute(
                "AllGather", ALU.bypass,
                replica_groups=[list(range(CORES))],
                ins=[h1s_d[:].opt()], outs=[h1f_d[:].opt()])

            if phase == "B":
                nc.sync.dma_start(dbgd[:], h1f_d[:])

            # ================= stage 2: GraphConv =================
            with tc.tile_pool(name="p2", bufs=3) as p2, \
                 tc.tile_pool(name="p2b", bufs=1) as p2b, \
                 tc.tile_pool(name="ps2", bufs=2, space="PSUM") as ps2, \
                 tc.tile_pool(name="ps2h", bufs=1, space="PSUM") as ps2h:
                h1blk = p2b.tile([128, KB, 2, H], f16, tag="h1blk")
                nc.sync.dma_start(
                    h1blk[:], h1f_d[:].rearrange(
                        "(kb p) (pr h) -> p kb pr h", p=128, pr=2))
                agg2_ps = ps2h.tile([128, NPC], f32, tag="agg2_ps")
                SC = 2
                for c0 in range(0, KB, SC):
                    s2t = p2.tile([128, SC, NPC], f16, tag="s2t")
                    nc.sync.dma_start(
                        s2t[:],
                        sel2d[:, c0 * NPC:(c0 + SC) * NPC].rearrange(
                            "p (b d) -> p b d", d=NPC))
                    for j in range(SC):
                        kb = c0 + j
                        for pr in range(2):
                            for hh in range(2):
                                hsl = slice(hh * 512, (hh + 1) * 512)
                                nc.tensor.matmul(
                                    agg2_ps[0:100, hsl],
                                    h1blk[:, kb, pr, :],
                                    s2t[:, j, hsl],
                                    start=(kb == 0 and pr == 0),
                                    stop=(kb == KB - 1 and pr == 1))
                agg2r = p2b.tile([100, NPC], f32, tag="agg2r")
                nc.vector.tensor_copy(agg2r[:], agg2_ps[0:100, :])
                h2T_ps = ps2h.tile([128, NPC], f32, tag="h2T_ps")
                for hh in range(2):
                    hsl = slice(hh * 512, (hh + 1) * 512)
                    nc.tensor.matmul(h2T_ps[0:100, hsl], gcrelr[:],
                                     agg2r[:, hsl], start=True, stop=False)
                    nc.tensor.matmul(h2T_ps[0:100, hsl], gcrootr[:],
                                     h1Tr[:, hsl], start=False, stop=False)
                    nc.tensor.matmul(h2T_ps[0:100, hsl], gcrelbr[:],
                                     onesr[:], start=False, stop=True)
                nc.vector.tensor_copy(h2Tr[:], h2T_ps[0:100, :])
                h2sb32 = p2b.tile([100, NPC], f32, tag="h2sb32")
                nc.vector.tensor_copy(h2sb32[:], h2T_ps[0:100, :])
                h2n = p2b.tile([128, 8, H], f32, tag="h2n")
                for t in range(8):
                    tp = ps2.tile([128, H], f32, tag="tp2")
                    nc.tensor.matmul(tp[:], h2sb32[:, t * 128:(t + 1) * 128],
                                     id32[0:100, 0:100], is_transpose=True,
                                     start=True, stop=True)
                    nc.vector.tensor_copy(h2n[:, t, :], tp[:])
                nc.sync.dma_start(
                    h2s_d[:].rearrange("(b p) h -> p b h", p=128), h2n[:])

            nc.gpsimd.collective_compute(
                "AllGather", ALU.bypass,
                replica_groups=[list(range(CORES))],
                ins=[h2s_d[:].opt()], outs=[h2f_d[:].opt()])

            # ---- before = emotions @ beta_w + b (own strip; overlaps AG2)
            with tc.tile_pool(name="psb", bufs=2, space="PSUM") as psb:
                emoTs = (xTsr[:, 0, :], xTsr[:, 1, :], h2Tr[:])
                for gc in range(3):
                    bps = psb.tile([100, NPC], f32, tag="bps")
                    for hh in range(2):
                        hsl = slice(hh * 512, (hh + 1) * 512)
                        for fc in range(3):
                            nc.tensor.matmul(bps[:, hsl],
                                             betawr[:, fc, gc, :],
                                             emoTs[fc][:, hsl],
                                             start=(fc == 0), stop=False)
                        nc.tensor.matmul(bps[:, hsl],
                                         betabr[:, gc * H:(gc + 1) * H],
                                         onesr[:], start=False, stop=True)
                    if gc < 2:
                        nc.vector.tensor_copy(bef16x[:, gc, :], bps[:])
                    else:
                        nc.vector.tensor_copy(bef2[:, 0, :], bps[:])
                        nc.vector.tensor_tensor(bef2[:, 1, :], bps[:],
                                                bef2[:, 0, :],
                                                op=ALU.subtract)
            pm.__exit__(None, None, None)

            # ================= keys + V + attention + head ===============
            pk = tc.tile_pool(name="pk", bufs=1)
            pkp = pk.__enter__()
            keysx = pkp.tile([100, 2, N], f16, tag="keysx")
            nc.sync.dma_start(keysx[:, 0, :], xT16d[0:100, :])
            nc.sync.dma_start(keysx[:, 1, :], xT16d[100:200, :])
            emoV = pkp.tile([128, KB, 300], f16, tag="emoV")
            nc.sync.dma_start(
                emoV[:, :, 0:F], xvd[:].rearrange("p (kb f) -> p kb f", f=F))
            keys2 = pkp.tile([100, 2, N], f16, tag="keys2")  # hi/lo
            if True:
                # h2 keys: f32 transposes of the AllGathered h2
                with tc.tile_pool(name="pkc", bufs=2) as pkc, \
                     tc.tile_pool(name="psk", bufs=2, space="PSUM") as psk:
                    for t0 in range(0, KB, 8):
                        h2fb = pkc.tile([128, 8, H], f32, tag="h2fb")
                        nc.sync.dma_start(
                            h2fb[:],
                            h2f_d[t0 * 128:(t0 + 8) * 128, :].rearrange(
                                "(kb p) h -> p kb h", p=128))
                        nc.vector.tensor_copy(emoV[:, t0:t0 + 8, F:300],
                                              h2fb[:])
                        for t in range(8):
                            tp = psk.tile([100, 128], f32, tag="tpk")
                            nc.tensor.matmul(tp[:], h2fb[:, t, :], id32[:],
                                             is_transpose=True,
                                             start=True, stop=True)
                            ks = slice((t0 + t) * 128, (t0 + t + 1) * 128)
                            nc.vector.tensor_copy(keys2[:, 0, ks], tp[:])
                            nc.vector.tensor_tensor(keys2[:, 1, ks], tp[:],
                                                    keys2[:, 0, ks],
                                                    op=ALU.subtract)

                if phase == "C":
                    nc.sync.dma_start(dbgd[:, 0:H],
                                      h1f_d[:].rearrange(
                                          "n (pr h) -> n pr h", pr=2)[:, 0, :])

                # ---- attention: q-tiles of 128, true row-max softmax
                with tc.tile_pool(name="pq", bufs=1) as pq, \
                     tc.tile_pool(name="pq2", bufs=2) as pq2, \
                     tc.tile_pool(name="pl", bufs=3) as pl, \
                     tc.tile_pool(name="pse", bufs=2, space="PSUM") as pse, \
                     tc.tile_pool(name="psp", bufs=2, space="PSUM") as psp, \
                     tc.tile_pool(name="pss", bufs=3, space="PSUM") as pss:
                    for qt in range(8):
                        qsl = slice(qt * 128, (qt + 1) * 128)
                        srow = pq.tile([128, N], f32, tag="srow")
                        mxc = pl.tile([128, 16], f32, tag="mxc")
                        for kt in range(16):
                            ksl = slice(kt * 512, (kt + 1) * 512)
                            sps = pss.tile([128, 512], f32, tag="sps")
                            for fc in range(2):
                                nc.tensor.matmul(sps[:], bef16x[:, fc, qsl],
                                                 keysx[:, fc, ksl],
                                                 start=(fc == 0), stop=False)
                            nc.tensor.matmul(sps[:], bef2[:, 0, qsl],
                                             keys2[:, 0, ksl],
                                             start=False, stop=False)
                            nc.tensor.matmul(sps[:], bef2[:, 0, qsl],
                                             keys2[:, 1, ksl],
                                             start=False, stop=False)
                            nc.tensor.matmul(sps[:], bef2[:, 1, qsl],
                                             keys2[:, 0, ksl],
                                             start=False, stop=True)
                            nc.scalar.activation(srow[:, ksl], sps[:],
                                                 AF.Copy)
                            nc.vector.reduce_max(mxc[:, kt:kt + 1], sps[:],
                                                 axis=AX.XYZW)
                        mx = pl.tile([128, 1], f32, tag="mx")
                        nc.vector.reduce_max(mx[:], mxc[:], axis=AX.XYZW)
                        nmx = pl.tile([128, 1], f32, tag="nmx")
                        nc.vector.tensor_scalar_mul(nmx[:], mx[:], -1.0)
                        prow = pq2.tile([128, N], f16, tag="prow")
                        nc.scalar.activation(prow[:], srow[:], AF.Exp,
                                             bias=nmx[:], scale=1.0,
                                             accum_out=rsum[:, qt:qt + 1])
                        em2_ps = pse.tile([128, 300], f32, tag="em2_ps")
                        for kb in range(KB):
                            ptp = psp.tile([128, 128], f16, tag="ptp")
                            nc.tensor.matmul(ptp[:],
                                             prow[:, kb * 128:(kb + 1) * 128],
                                             id16[:], is_transpose=True,
                                             start=True, stop=True)
                            pts = pl.tile([128, 128], f16, tag="pts")
                            nc.vector.tensor_copy(pts[:], ptp[:])
                            nc.tensor.matmul(em2_ps[:], pts[:],
                                             emoV[:, kb, :],
                                             start=(kb == 0),
                                             stop=(kb == KB - 1))
                        nc.vector.tensor_copy(em2sb[:, qt, :], em2_ps[:])
            pk.__exit__(None, None, None)

            # ================= head =================
            with tc.tile_pool(name="ph", bufs=2) as ph, \
                 tc.tile_pool(name="psh", bufs=1, space="PSUM") as psh:
                for qt in range(8):
                    rcp = ph.tile([128, 1], f32, tag="rcp")
                    nc.vector.reciprocal(rcp[:], rsum[:, qt:qt + 1])
                    em2n = ph.tile([128, 3, H], f16, tag="em2n")
                    nc.vector.tensor_scalar(em2n[:],
                                            em2sb[:, qt, :].rearrange(
                                                "p (c h) -> p c h", c=3),
                                            rcp[:], None, op0=ALU.mult)
                    e2t = ph.tile([H, 3, 128], f16, tag="e2t")
                    for c in range(3):
                        tp = psh.tile([H, 128], f16, tag="tpb")
                        nc.tensor.matmul(tp[:], em2n[:, c, :],
                                         id16[:], is_transpose=True,
                                         start=True, stop=True)
                        nc.vector.tensor_copy(e2t[:, c, :], tp[:])
                    hid_ps = psh.tile([H, 128], f32, tag="hid_ps")
                    for c in range(3):
                        nc.tensor.matmul(hid_ps[:], linw[:, c, :],
                                         e2t[:, c, :],
                                         start=(c == 0), stop=False)
                    nc.tensor.matmul(hid_ps[:], linb[:], ones16[:, 0:128],
                                     start=False, stop=True)
                    hidT = ph.tile([H, 128], f16, tag="hidT")
                    nc.scalar.activation(hidT[:], hid_ps[:], AF.Relu)
                    lg_ps = psh.tile([NC, 128], f32, tag="lg_ps")
                    nc.tensor.matmul(lg_ps[:], smaxw[:], hidT[:],
                                     start=True, stop=False)
                    nc.tensor.matmul(lg_ps[:], smaxb[:], ones16[:, 0:128],
                                     start=False, stop=True)
                    lgT = ph.tile([NC, 128], f32, tag="lgT")
                    nc.vector.tensor_copy(lgT[:], lg_ps[:])
                    plt = psh.tile([128, NC], f32, tag="plt")
                    nc.tensor.matmul(plt[:], lgT[:], id32[0:NC, 0:NC],
                                     is_transpose=True, start=True, stop=True)
                    lg = ph.tile([128, NC], f32, tag="lg")
                    nc.vector.tensor_copy(lg[:], plt[:])
                    m6 = ph.tile([128, 1], f32, tag="m6")
                    nc.vector.reduce_max(m6[:], lg[:], axis=AX.XYZW)
                    nm6 = ph.tile([128, 1], f32, tag="nm6")
                    nc.vector.tensor_scalar_mul(nm6[:], m6[:], -1.0)
                    e6 = ph.tile([128, NC], f32, tag="e6")
                    s6 = ph.tile([128, 1], f32, tag="s6")
                    nc.scalar.activation(e6[:], lg[:], AF.Exp, bias=nm6[:],
                                         scale=1.0, accum_out=s6[:])
                    ls6 = ph.tile([128, 1], f32, tag="ls6")
                    nc.scalar.activation(ls6[:], s6[:], AF.Ln)
                    sh = ph.tile([128, 1], f32, tag="sh")
                    nc.vector.tensor_add(sh[:], m6[:], ls6[:])
                    outt = ph.tile([128, NC], f32, tag="outt")
                    nc.vector.tensor_scalar(outt[:], lg[:], sh[:], None,
                                            op0=ALU.subtract)
                    nc.sync.dma_start(outd[qt * 128:(qt + 1) * 128, :],
                                      outt[:])

    nc.compile()
    return nc


# ------------------------------------------------------------------ entry
def kernel(x, edge_index, edge_norm, edge_type, basis, comp, root_w, root_b,
           gc_rel_w, gc_rel_b, gc_root_w, beta_w, beta_b, lin_w, lin_b,
           smax_w, smax_b):
    x = np.ascontiguousarray(np.asarray(x, np.float32))
    per_core, meta = _prep(x, edge_index, edge_type)

    import os
    phase = os.environ.get("KPHASE", "full")
    key = (phase, meta["totB1"], tuple(meta["B1"]))
    if key not in _ker_cache:
        _ker_cache[key] = _build(meta, phase)
    nc = _ker_cache[key]

    x16 = x.astype(np.float16)
    xT = np.ascontiguousarray(x.T)
    linw_pack = np.ascontiguousarray(
        np.asarray(lin_w, np.float16).reshape(3, 100, H).transpose(
            1, 0, 2)).reshape(100, 3 * H)
    bw = np.asarray(beta_w, np.float32)           # [300, 300]
    betawT = np.ascontiguousarray(
        bw.reshape(3, 100, 3, 100).transpose(1, 0, 2, 3)).reshape(100, -1)
    rootw32 = np.ascontiguousarray(
        np.asarray(root_w, np.float32).reshape(2, 100, H).transpose(
            1, 0, 2)).reshape(100, 2 * H)

    shared = dict(
        xT16=np.ascontiguousarray(x16.T),
        xv=np.ascontiguousarray(
            x16.reshape(KB, 128, F).transpose(1, 0, 2)).reshape(128, KB * F),
        basis32=np.ascontiguousarray(
            np.asarray(basis, np.float32).reshape(NB, F * H)),
        compT32=np.ascontiguousarray(np.asarray(comp, np.float32).T),
        rootw32=rootw32,
        rootb32=np.asarray(root_b, np.float32).reshape(1, H),
        gcrel32=np.asarray(gc_rel_w, np.float32),
        gcrelb32=np.asarray(gc_rel_b, np.float32).reshape(1, H),
        gcroot32=np.asarray(gc_root_w, np.float32),
        betaw32=betawT,
        betab32=np.asarray(beta_b, np.float32).reshape(1, 3 * H),
        linw16=linw_pack,
        linb16=np.asarray(lin_b, np.float16).reshape(1, H),
        smaxw16=np.asarray(smax_w, np.float16),
        smaxb16=np.asarray(smax_b, np.float16).reshape(1, NC),
        ident16=np.eye(128, dtype=np.float16),
        ident32=np.eye(128, dtype=np.float32),
        ones32=np.ones((1, 512), np.float32),
        ones16=np.ones((1, NPC), np.float16),
    )
    in_maps = []
    for c in range(CORES):
        m = dict(shared)
        strip = xT[:, c * NPC:(c + 1) * NPC]
        m["xTs32"] = np.ascontiguousarray(
            strip.reshape(2, 100, NPC).transpose(1, 0, 2)).reshape(
            100, 2 * NPC)
        m.update(per_core[c])
        in_maps.append(m)

    res = run_bass_kernel_spmd(nc, in_maps, core_ids=list(range(CORES)),
                               trace_cores=[0])
    global _last_res
    _last_res = res
    if phase != "full":
        return [res.results[c]["dbg"] for c in range(CORES)]
    return np.concatenate([res.results[c]["out"] for c in range(CORES)],
                          axis=0)



# revision 25
# speedup vs baseline: 1.0186x; 1.0186x over previous
"""DialogueGCN forward on 8 Trainium2 NeuronCores (Bass/Tile).

kernel(**inputs) -> np.ndarray [8192, 6] log-probs, matching reference().

Sharding: nodes row-sharded 1024/core; each core owns edges into its dst
strip. Stage-1 edge features are host-permuted x rows (xg, fp16) streamed
contiguously and segment-summed into (dst,rel) buckets via narrow selection
matmuls, then transformed with W (linearity of RGCN). Stage-2 reads the
AllGathered h1 and aggregates with a dense count matrix.

v2 structure (vs baseline):
- h1 AllGather split hi/lo fp16 so stage-2's hi half overlaps the lo AG;
  sel2 partially preloaded to SBUF during stage-1/AG1 so stage-2 is
  compute-bound and starts right after AG-hi.
- h2 is AllGathered TRANSPOSED (hi/lo fp16 strips) -> keys2 needs zero
  on-device transposes; node-major h2 is never materialized.
- attention V is emotions @ lin_w (100 wide, built once from SBUF keys via
  16x3 matmuls + one DMA-xbar transpose); P is transposed per q-tile with a
  single dma_start_transpose instead of 64 TensorE transposes + copies.
- x-part of `before` computed during AG1; h2 part added after stage-2.
- weight/preload DMAs ride the scalar HWDGE queue so the sync queue streams
  xg/sel without head-of-line blocking.

Precision: identical to baseline — S-feeding chain at f32 / fp16-hi+lo
(h1, h2 cross cores as fp16 hi+lo pairs; before/W/transform in f32;
softmax with true per-row max).
"""
import numpy as np

import concourse.bass as bass
import concourse.tile as tile
import concourse.mybir as mybir
from concourse import bacc
from concourse.bass_utils import run_bass_kernel_spmd

f32 = mybir.dt.float32
f16 = mybir.dt.float16

N, E, F, H, R, NB, NC = 8192, 680000, 200, 100, 8, 30, 6
CORES = 8
NPC = N // CORES            # 1024 dst rows per core
NG = NPC // 32              # 32-dst groups per core (32)
NBIN = NG * R               # bins per core (256)
KB = N // 128               # key blocks (64)
S2P = 8                     # sel2 kb-blocks preloaded to SBUF

AF = mybir.ActivationFunctionType
ALU = mybir.AluOpType
AX = mybir.AxisListType

_ker_cache = {}
_last_res = None


# ------------------------------------------------------------------ host prep
def _prep(x, edge_index, edge_type):
    src = np.asarray(edge_index[0], np.int64)
    dst = np.asarray(edge_index[1], np.int64)
    et = np.asarray(edge_type, np.int64)

    deg = np.bincount(dst * R + et, minlength=N * R).astype(np.float64)
    inv = np.where(deg > 0, 1.0 / np.maximum(deg, 1.0), 0.0).astype(np.float32)
    invv = inv[dst * R + et]                       # per-edge weight

    core = dst >> 10
    g = (dst & 1023) >> 5
    binid = ((core * NG + g) << 3) | et            # [0, CORES*NBIN)
    order = np.argsort(binid, kind="stable")
    cnt = np.bincount(binid, minlength=CORES * NBIN).reshape(CORES, NBIN)
    B1 = (-(-cnt // 128)).max(axis=0)              # blocks per bin (static)
    totB1 = int(B1.sum())
    bstart = np.concatenate([[0], np.cumsum(B1)])  # block offset per bin

    flat_cnt = cnt.reshape(-1)
    starts = np.concatenate([[0], np.cumsum(flat_cnt)])
    pos = np.arange(E, dtype=np.int64) - np.repeat(starts[:-1], flat_cnt)
    sbin = binid[order]
    blk_all = bstart[sbin % NBIN] + (pos >> 7)
    row_all = pos & 127

    x16 = np.asarray(x, np.float32).astype(np.float16)
    per_core = []
    for c in range(CORES):
        sl = slice(starts[c * NBIN], starts[(c + 1) * NBIN])
        e = order[sl]
        bl = blk_all[sl]
        rw = row_all[sl]
        xg = np.zeros((128, totB1, F), np.float16)
        xg[rw, bl] = x16[src[e]]
        sel1 = np.zeros((128, totB1, 32), np.float16)
        sel1[rw, bl, dst[e] & 31] = invv[e]
        cnt2 = np.bincount(src[e] * NPC + (dst[e] & 1023), minlength=N * NPC)
        sel2 = np.ascontiguousarray(
            cnt2.reshape(KB, 128, NPC).transpose(1, 0, 2)).astype(np.float16)
        per_core.append(dict(
            xg=xg.reshape(128, totB1 * F),
            sel1=sel1.reshape(128, totB1 * 32),
            sel2=sel2.reshape(128, KB * NPC)))

    meta = dict(B1=[int(b) for b in B1], totB1=totB1)
    return per_core, meta


# ------------------------------------------------------------------ program
def _build(meta, phase="full"):
    B1 = meta["B1"]
    totB1 = meta["totB1"]

    nc = bacc.Bacc("TRN2", target_bir_lowering=False, debug=False,
                   num_devices=CORES)
    P = lambda n, s, d: nc.declare_dram_parameter(n, s, d, isOutput=False)

    xT16d = P("xT16", [F, N], f16)             # keys x-part
    xTs32d = P("xTs32", [100, 2 * NPC], f32)   # own strip xT, f32
    basis32d = P("basis32", [NB, F * H], f32)
    compT32d = P("compT32", [NB, R], f32)
    rootw32d = P("rootw32", [100, 2 * H], f32)
    rootb32d = P("rootb32", [1, H], f32)
    gcrel32d = P("gcrel32", [H, H], f32)
    gcrelb32d = P("gcrelb32", [1, H], f32)
    gcroot32d = P("gcroot32", [H, H], f32)
    betaw32d = P("betaw32", [100, 3 * 3 * H], f32)  # [f, fc, gc, h]
    betab32d = P("betab32", [1, 3 * H], f32)
    linwd = P("linw16", [100, 3 * H], f16)     # chunks [f, c, h]
    linb32d = P("linb32", [1, H], f32)
    smaxwd = P("smaxw16", [H, NC], f16)
    smaxbd = P("smaxb16", [1, NC], f16)
    id16d = P("ident16", [128, 128], f16)
    id32d = P("ident32", [128, 128], f32)
    ones32d = P("ones32", [1, 512], f32)
    ones16d = P("ones16", [1, NPC], f16)
    xgd = P("xg", [128, totB1 * F], f16)
    sel1d = P("sel1", [128, totB1 * 32], f16)
    sel2d = P("sel2", [128, KB * NPC], f16)

    outd = nc.declare_dram_parameter("out", [NPC, NC], f32, isOutput=True)
    dbgd = None
    if phase != "full":
        dbgd = nc.declare_dram_parameter("dbg", [N, 2 * H], f16, isOutput=True)

    with tile.TileContext(nc, num_cores=CORES) as tc:
        with tc.tile_pool(name="dram", bufs=1, space="DRAM") as dram, \
             tc.tile_pool(name="persist", bufs=1) as pp:

            wtmp_d = dram.tile([R, F * H], f32, tag="wtmp")
            h1hs_d = dram.tile([NPC, H], f16, tag="h1hs")
            h1ls_d = dram.tile([NPC, H], f16, tag="h1ls")
            h1hf_d = dram.tile([N, H], f16, tag="h1hf")
            h1lf_d = dram.tile([N, H], f16, tag="h1lf")
            h2Ts_d = dram.tile([100, 2 * NPC], f16, tag="h2Ts")
            h2Tf_d = dram.tile([CORES * 100, 2 * NPC], f16, tag="h2Tf")

            # --------- persistent SBUF (small; DMAs emitted later) ---------
            id16 = pp.tile([128, 128], f16, tag="id16")
            id32 = pp.tile([128, 128], f32, tag="id32")
            ones16 = pp.tile([1, NPC], f16, tag="ones16")
            linw = pp.tile([100, 3, H], f16, tag="linw")
            smaxw = pp.tile([H, NC], f16, tag="smaxw")
            smaxb = pp.tile([1, NC], f16, tag="smaxb")
            linb_bc = pp.tile([128, H], f32, tag="linb_bc")

            # f32 weights for the S-precision chain (whole-kernel lifetime)
            pm = tc.tile_pool(name="pm", bufs=1)
            pmp = pm.__enter__()
            onesr = pmp.tile([1, 512], f32, tag="onesr")
            gcrelr = pmp.tile([H, H], f32, tag="gcrelr")
            gcrelbr = pmp.tile([1, H], f32, tag="gcrelbr")
            gcrootr = pmp.tile([H, H], f32, tag="gcrootr")
            betawr = pmp.tile([100, 3, 3, H], f32, tag="betawr")
            betabr = pmp.tile([1, 3 * H], f32, tag="betabr")
            xTsr = pmp.tile([100, 2, NPC], f32, tag="xTsr")
            linb32 = pmp.tile([1, H], f32, tag="linb32")
            h1Tr = pmp.tile([100, NPC], f32, tag="h1Tr")
            h2Tr = pmp.tile([100, NPC], f32, tag="h2Tr")

            # ================= phase W: relation weights =================
            with tc.tile_pool(name="pw", bufs=2) as pw, \
                 tc.tile_pool(name="pwc", bufs=3) as pwc, \
                 tc.tile_pool(name="psw", bufs=2, space="PSUM") as psw:
                compT = pw.tile([NB, R], f32, tag="compT")
                nc.sync.dma_start(compT[:], compT32d[:])
                CB = 2500
                for cb in range(F * H // CB):
                    bchunk = pw.tile([NB, CB], f32, tag="bchunk")
                    nc.sync.dma_start(
                        bchunk[:], basis32d[:, cb * CB:(cb + 1) * CB])
                    for t in range(CB // 500):
                        pwp = psw.tile([R, 500], f32, tag="pwp")
                        nc.tensor.matmul(pwp[:], compT[:],
                                         bchunk[:, t * 500:(t + 1) * 500],
                                         start=True, stop=True)
                        wsb = pwc.tile([R, 500], f32, tag="wsb")
                        nc.vector.tensor_copy(wsb[:], pwp[:])
                        nc.sync.dma_start(
                            wtmp_d[:, cb * CB + t * 500:
                                   cb * CB + (t + 1) * 500], wsb[:])

            # background loads on the scalar HWDGE queue
            nc.scalar.dma_start(id16[:], id16d[:])
            nc.scalar.dma_start(id32[:], id32d[:])
            nc.scalar.dma_start(ones16[:], ones16d[:])
            nc.scalar.dma_start(onesr[:], ones32d[:])
            nc.scalar.dma_start(gcrelr[:], gcrel32d[:])
            nc.scalar.dma_start(gcrelbr[:], gcrelb32d[:])
            nc.scalar.dma_start(gcrootr[:], gcroot32d[:])
            nc.scalar.dma_start(betawr[:], betaw32d[:].rearrange(
                "p (fc gc h) -> p fc gc h", fc=3, gc=3))
            nc.scalar.dma_start(betabr[:], betab32d[:])
            nc.scalar.dma_start(xTsr[:], xTs32d[:].rearrange(
                "p (c n) -> p c n", c=2))
            nc.scalar.dma_start(linb32[:], linb32d[:])
            nc.scalar.dma_start(linw[:], linwd[:].rearrange(
                "p (c h) -> p c h", c=3))
            nc.scalar.dma_start(smaxw[:], smaxwd[:])
            nc.scalar.dma_start(smaxb[:], smaxbd[:])

            # ================= stage 1: RGCN =================
            CH = 16                          # blocks per DMA chunk
            with tc.tile_pool(name="p1", bufs=3) as p1, \
                 tc.tile_pool(name="p1b", bufs=2) as p1b, \
                 tc.tile_pool(name="p1c", bufs=1) as p1c, \
                 tc.tile_pool(name="ps1", bufs=2, space="PSUM") as ps1, \
                 tc.tile_pool(name="ps1h", bufs=1, space="PSUM") as ps1h:
                aggsbr = p1c.tile([128, 2, R, NPC], f32, tag="aggsbr")
                h1T_ps = ps1h.tile([128, NPC], f32, tag="h1T_ps")
                Wtr = p1c.tile([100, 2, R, H], f32, tag="Wtr")
                rootwr = p1c.tile([100, 2, H], f32, tag="rootwr")
                rootbr = p1c.tile([1, H], f32, tag="rootbr")
                nc.scalar.dma_start(rootwr[:], rootw32d[:].rearrange(
                    "p (c h) -> p c h", c=2))
                nc.scalar.dma_start(rootbr[:], rootb32d[:])
                # W readback transposed: Wt[fp,fc,r,h] = W[r, fc*100+fp, h]
                for fc in range(2):
                    nc.scalar.dma_start(
                        Wtr[:, fc, :, :],
                        wtmp_d[:, fc * 100 * H:(fc + 1) * 100 * H].rearrange(
                            "r (fp h) -> fp r h", h=H))

                binof = []
                for b_idx, nb in enumerate(B1):
                    binof += [b_idx] * nb
                gfirst = {}
                glast = {}
                for b in range(totB1):
                    gg = binof[b] >> 3
                    if gg not in gfirst:
                        gfirst[gg] = b
                    glast[gg] = b
                agg_ps = None
                gcur = -1
                bo = 0
                while bo < totB1:
                    k = min(CH, totB1 - bo)
                    xgt = p1.tile([128, CH, F], f16, tag="xgt")
                    nc.sync.dma_start(
                        xgt[:, 0:k, :],
                        xgd[:, bo * F:(bo + k) * F].rearrange(
                            "p (b f) -> p b f", f=F))
                    selt = p1.tile([128, CH, 32], f16, tag="selt")
                    nc.sync.dma_start(
                        selt[:, 0:k, :],
                        sel1d[:, bo * 32:(bo + k) * 32].rearrange(
                            "p (b d) -> p b d", d=32))
                    for j in range(k):
                        b = bo + j
                        bn = binof[b]
                        gg, rr = bn >> 3, bn & 7
                        if gg != gcur:
                            if agg_ps is not None:
                                nc.vector.tensor_copy(
                                    aggsbr[:, :, :, gcur * 32:(gcur + 1) * 32],
                                    agg_ps[:].rearrange(
                                        "p fc (r d) -> p fc r d", d=32))
                            agg_ps = ps1.tile([128, 2, R * 32], f32,
                                              tag="agg_ps")
                            gcur = gg
                        for fc in range(2):
                            nc.tensor.matmul(
                                agg_ps[0:100, fc, rr * 32:(rr + 1) * 32],
                                xgt[:, j, fc * 100:(fc + 1) * 100],
                                selt[:, j, :],
                                start=(b == gfirst[gg] and fc == 0),
                                stop=(b == glast[gg] and fc == 1))
                    bo += k
                nc.vector.tensor_copy(
                    aggsbr[:, :, :, gcur * 32:(gcur + 1) * 32],
                    agg_ps[:].rearrange("p fc (r d) -> p fc r d", d=32))

                # transform: h1T = sum_r W_r^T agg_r + root + bias (f32)
                for hh in range(2):
                    hsl = slice(hh * 512, (hh + 1) * 512)
                    mmi = 0
                    for fc in range(2):
                        for rr in range(R):
                            nc.tensor.matmul(h1T_ps[0:100, hsl],
                                             Wtr[:, fc, rr, :],
                                             aggsbr[0:100, fc, rr, hsl],
                                             start=(mmi == 0), stop=False)
                            mmi += 1
                    for fc in range(2):
                        nc.tensor.matmul(h1T_ps[0:100, hsl], rootwr[:, fc, :],
                                         xTsr[:, fc, hsl],
                                         start=False, stop=False)
                    nc.tensor.matmul(h1T_ps[0:100, hsl], rootbr[:],
                                     onesr[:], start=False, stop=True)
                nc.vector.tensor_copy(h1Tr[:], h1T_ps[0:100, :])

                # h1 -> fp16 hi + lo pair, node-major strips, DRAM
                h1hi = p1b.tile([100, NPC], f16, tag="h1hi")
                nc.vector.tensor_copy(h1hi[:], h1T_ps[0:100, :])
                h1lo = p1b.tile([100, NPC], f16, tag="h1lo")
                nc.vector.tensor_tensor(h1lo[:], h1T_ps[0:100, :], h1hi[:],
                                        op=ALU.subtract)
                h1n = p1b.tile([128, 8, 2, H], f16, tag="h1n")
                for t in range(8):
                    for pr, part in enumerate((h1hi, h1lo)):
                        tp = ps1.tile([128, H], f16, tag="tp1")
                        nc.tensor.matmul(tp[:], part[:, t * 128:(t + 1) * 128],
                                         id16[0:100, 0:100], is_transpose=True,
                                         start=True, stop=True)
                        nc.vector.tensor_copy(h1n[:, t, pr, :], tp[:])
                nc.sync.dma_start(
                    h1hs_d[:].rearrange("(b p) h -> p b h", p=128),
                    h1n[:, :, 0, :])
                nc.sync.dma_start(
                    h1ls_d[:].rearrange("(b p) h -> p b h", p=128),
                    h1n[:, :, 1, :])

            nc.gpsimd.collective_compute(
                "AllGather", ALU.bypass,
                replica_groups=[list(range(CORES))],
                ins=[h1hs_d[:].opt()], outs=[h1hf_d[:].opt()])
            nc.gpsimd.collective_compute(
                "AllGather", ALU.bypass,
                replica_groups=[list(range(CORES))],
                ins=[h1ls_d[:].opt()], outs=[h1lf_d[:].opt()])

            # attention persistents (allocated now; stage-1 pools freed)
            pa = tc.tile_pool(name="pa", bufs=1)
            pap = pa.__enter__()
            keysx = pap.tile([100, 2, N], f16, tag="keysx")
            keys2 = pap.tile([100, 2, 8, NPC], f16, tag="keys2")
            Vp = pap.tile([128, KB, 128], f16, tag="Vp")
            bef16x = pap.tile([100, 2, NPC], f16, tag="bef16x")
            bef2 = pap.tile([100, 2, NPC], f16, tag="bef2")
            rsum = pap.tile([128, 8], f32, tag="rsum")

            nc.scalar.dma_start(keysx[:, 0, :], xT16d[0:100, :])
            nc.scalar.dma_start(keysx[:, 1, :], xT16d[100:200, :])

            if phase == "B":
                nc.sync.dma_start(dbgd[:, 0:H], h1hf_d[:])
                nc.sync.dma_start(dbgd[:, H:2 * H], h1lf_d[:])

            # ================= stage 2: GraphConv =================
            with tc.tile_pool(name="p2", bufs=2) as p2, \
                 tc.tile_pool(name="p2b", bufs=1) as p2b, \
                 tc.tile_pool(name="psbx", bufs=1, space="PSUM") as psbx, \
                 tc.tile_pool(name="ps2h", bufs=1, space="PSUM") as ps2h:
                s2a = p2b.tile([128, S2P, NPC], f16, tag="s2a")
                befxp = p2b.tile([100, 3, NPC], f32, tag="befxp")
                # sel2 preload (lands during stage-1 tail / AG1)
                for c0 in range(0, S2P, 4):
                    nc.scalar.dma_start(
                        s2a[:, c0:c0 + 4, :],
                        sel2d[:, c0 * NPC:(c0 + 4) * NPC].rearrange(
                            "p (b d) -> p b d", d=NPC))

                # before: x-part + linb broadcast (runs during AG1)
                for gc in range(3):
                    bx = psbx.tile([100, NPC], f32, tag="bx")
                    for hh in range(2):
                        hsl = slice(hh * 512, (hh + 1) * 512)
                        for fc in range(2):
                            nc.tensor.matmul(bx[:, hsl],
                                             betawr[:, fc, gc, :],
                                             xTsr[:, fc, hsl],
                                             start=(fc == 0), stop=False)
                        nc.tensor.matmul(bx[:, hsl],
                                         betabr[:, gc * H:(gc + 1) * H],
                                         onesr[:], start=False, stop=True)
                    nc.vector.tensor_copy(befxp[:, gc, :], bx[:])
                lbp = psbx.tile([128, H], f32, tag="lbp")
                nc.tensor.matmul(lbp[:], onesr[:, 0:128], linb32[:],
                                 start=True, stop=True)
                nc.vector.tensor_copy(linb_bc[:], lbp[:])

                h1bh = p2b.tile([128, KB, H], f16, tag="h1bh")
                nc.sync.dma_start(
                    h1bh[:], h1hf_d[:].rearrange("(kb p) h -> p kb h", p=128))
                h1bl = p2b.tile([128, KB, H], f16, tag="h1bl")
                nc.sync.dma_start(
                    h1bl[:], h1lf_d[:].rearrange("(kb p) h -> p kb h", p=128))
                agg2_ps = ps2h.tile([128, NPC], f32, tag="agg2_ps")

                def s2mm(lhsT, rhs_tile, kb_local, first, last):
                    for hh in range(2):
                        hsl = slice(hh * 512, (hh + 1) * 512)
                        nc.tensor.matmul(
                            agg2_ps[0:100, hsl], lhsT,
                            rhs_tile[:, kb_local, hsl],
                            start=first, stop=last)

                # phase A: hi x preloaded blocks (overlaps AG1-lo)
                for kb in range(S2P):
                    s2mm(h1bh[:, kb, :], s2a, kb, kb == 0, False)
                # phase B: stream remaining blocks, hi+lo per chunk
                SC = 4
                for c0 in range(S2P, KB, SC):
                    s2t = p2.tile([128, SC, NPC], f16, tag="s2t")
                    nc.scalar.dma_start(
                        s2t[:],
                        sel2d[:, c0 * NPC:(c0 + SC) * NPC].rearrange(
                            "p (b d) -> p b d", d=NPC))
                    for j in range(SC):
                        kb = c0 + j
                        s2mm(h1bh[:, kb, :], s2t, j, False, False)
                        s2mm(h1bl[:, kb, :], s2t, j, False, False)
                # phase C: lo x preloaded blocks
                for kb in range(S2P):
                    s2mm(h1bl[:, kb, :], s2a, kb, False, kb == S2P - 1)

                agg2r = p2b.tile([100, NPC], f32, tag="agg2r")
                nc.vector.tensor_copy(agg2r[:], agg2_ps[0:100, :])
                h2T_ps = ps2h.tile([128, NPC], f32, tag="h2T_ps")
                for hh in range(2):
                    hsl = slice(hh * 512, (hh + 1) * 512)
                    nc.tensor.matmul(h2T_ps[0:100, hsl], gcrelr[:],
                                     agg2r[:, hsl], start=True, stop=False)
                    nc.tensor.matmul(h2T_ps[0:100, hsl], gcrootr[:],
                                     h1Tr[:, hsl], start=False, stop=False)
                    nc.tensor.matmul(h2T_ps[0:100, hsl], gcrelbr[:],
                                     onesr[:], start=False, stop=True)
                nc.vector.tensor_copy(h2Tr[:], h2T_ps[0:100, :])

                # h2^T -> fp16 hi + lo strip, DRAM (no transposes needed)
                h2Tn = p2b.tile([100, 2, NPC], f16, tag="h2Tn")
                nc.vector.tensor_copy(h2Tn[:, 0, :], h2T_ps[0:100, :])
                nc.vector.tensor_tensor(h2Tn[:, 1, :], h2T_ps[0:100, :],
                                        h2Tn[:, 0, :], op=ALU.subtract)
                nc.sync.dma_start(
                    h2Ts_d[:].rearrange("p (pr n) -> p pr n", pr=2), h2Tn[:])

                nc.gpsimd.collective_compute(
                    "AllGather", ALU.bypass,
                    replica_groups=[list(range(CORES))],
                    ins=[h2Ts_d[:].opt()], outs=[h2Tf_d[:].opt()])

                # before: h2 part + combine (overlaps AG2)
                for gc in range(3):
                    bh = psbx.tile([100, NPC], f32, tag="bx")
                    for hh in range(2):
                        hsl = slice(hh * 512, (hh + 1) * 512)
                        nc.tensor.matmul(bh[:, hsl], betawr[:, 2, gc, :],
                                         h2Tr[:, hsl],
                                         start=True, stop=True)
                    if gc < 2:
                        nc.vector.tensor_tensor(bef16x[:, gc, :], bh[:],
                                                befxp[:, gc, :], op=ALU.add)
                    else:
                        b32 = p2b.tile([100, NPC], f32, tag="b32")
                        nc.vector.tensor_tensor(b32[:], bh[:],
                                                befxp[:, gc, :], op=ALU.add)
                        nc.vector.tensor_copy(bef2[:, 0, :], b32[:])
                        nc.vector.tensor_tensor(bef2[:, 1, :], b32[:],
                                                bef2[:, 0, :],
                                                op=ALU.subtract)

            if phase == "C":
                nc.sync.dma_start(
                    dbgd[:].rearrange("(c n) (pr p) -> (c p) (pr n)",
                                      n=NPC, p=100), h2Tf_d[:])

            # keys2 from transposed AG (no on-device transposes)
            nc.sync.dma_start(
                keys2[:], h2Tf_d[:].rearrange(
                    "(c p) (pr n) -> p pr c n", p=100, pr=2))

            # V' = emotions @ lin_w  [N, H] built as V'^T then xbar transpose
            with tc.tile_pool(name="pv", bufs=1) as pv, \
                 tc.tile_pool(name="pvc", bufs=3) as pvc, \
                 tc.tile_pool(name="psv", bufs=4, space="PSUM") as psv:
                VT = pv.tile([128, N], f16, tag="VT")
                nc.vector.memset(VT[:], 0.0)
                for ch in range(16):
                    csl = slice(ch * 512, (ch + 1) * 512)
                    vq = psv.tile([100, 512], f32, tag="vq")
                    nc.tensor.matmul(vq[:], linw[:, 0, :],
                                     keysx[:, 0, csl], start=True, stop=False)
                    nc.tensor.matmul(vq[:], linw[:, 1, :],
                                     keysx[:, 1, csl], start=False, stop=False)
                    nc.tensor.matmul(vq[:], linw[:, 2, :],
                                     keys2[:, 0, ch >> 1,
                                           (ch & 1) * 512:(ch & 1) * 512 + 512],
                                     start=False, stop=True)
                    vc = pvc.tile([100, 512], f16, tag="vc")
                    nc.vector.tensor_copy(vc[:], vq[:])
                    nc.vector.tensor_copy(VT[0:100, csl], vc[:])
                nc.sync.dma_start_transpose(Vp[:], VT[:])

            # ================= attention + head ===============
            with tc.tile_pool(name="pq", bufs=1) as pq, \
                 tc.tile_pool(name="pq2", bufs=2) as pq2, \
                 tc.tile_pool(name="pt", bufs=1) as pt, \
                 tc.tile_pool(name="pl", bufs=3) as pl, \
                 tc.tile_pool(name="pse", bufs=2, space="PSUM") as pse, \
                 tc.tile_pool(name="psh", bufs=1, space="PSUM") as psh, \
                 tc.tile_pool(name="pss", bufs=3, space="PSUM") as pss:
                for qt in range(8):
                    qsl = slice(qt * 128, (qt + 1) * 128)
                    srow = pq.tile([128, N], f32, tag="srow")
                    mxc = pl.tile([128, 16], f32, tag="mxc")
                    for kt in range(16):
                        ksl = slice(kt * 512, (kt + 1) * 512)
                        k2s = slice((kt & 1) * 512, (kt & 1) * 512 + 512)
                        sps = pss.tile([128, 512], f32, tag="sps")
                        for fc in range(2):
                            nc.tensor.matmul(sps[:], bef16x[:, fc, qsl],
                                             keysx[:, fc, ksl],
                                             start=(fc == 0), stop=False)
                        nc.tensor.matmul(sps[:], bef2[:, 0, qsl],
                                         keys2[:, 0, kt >> 1, k2s],
                                         start=False, stop=False)
                        nc.tensor.matmul(sps[:], bef2[:, 0, qsl],
                                         keys2[:, 1, kt >> 1, k2s],
                                         start=False, stop=False)
                        nc.tensor.matmul(sps[:], bef2[:, 1, qsl],
                                         keys2[:, 0, kt >> 1, k2s],
                                         start=False, stop=True)
                        nc.scalar.activation(srow[:, ksl], sps[:], AF.Copy)
                        nc.vector.reduce_max(mxc[:, kt:kt + 1], sps[:],
                                             axis=AX.XYZW)
                    mx = pl.tile([128, 1], f32, tag="mx")
                    nc.vector.reduce_max(mx[:], mxc[:], axis=AX.XYZW)
                    nmx = pl.tile([128, 1], f32, tag="nmx")
                    nc.vector.tensor_scalar_mul(nmx[:], mx[:], -1.0)
                    prow = pq2.tile([128, N], f16, tag="prow")
                    nc.scalar.activation(prow[:], srow[:], AF.Exp,
                                         bias=nmx[:], scale=1.0,
                                         accum_out=rsum[:, qt:qt + 1])
                    prowT = pt.tile([128, KB, 128], f16, tag="prowT")
                    nc.sync.dma_start_transpose(prowT[:], prow[:])
                    em2_ps = pse.tile([128, H], f32, tag="em2_ps")
                    for kb in range(KB):
                        nc.tensor.matmul(em2_ps[:], prowT[:, kb, :],
                                         Vp[:, kb, 0:H],
                                         start=(kb == 0), stop=(kb == KB - 1))
                    # hidden = relu(em2/rsum + lin_b)
                    rcp = pl.tile([128, 1], f32, tag="rcp")
                    nc.vector.reciprocal(rcp[:], rsum[:, qt:qt + 1])
                    em2f = pl.tile([128, H], f32, tag="em2f")
                    nc.vector.tensor_scalar(em2f[:], em2_ps[:], rcp[:], None,
                                            op0=ALU.mult)
                    emb = pl.tile([128, H], f32, tag="emb")
                    nc.vector.tensor_tensor(emb[:], em2f[:], linb_bc[:],
                                            op=ALU.add)
                    hid16 = pl.tile([128, H], f16, tag="hid16")
                    nc.scalar.activation(hid16[:], emb[:], AF.Relu)
                    tp = psh.tile([H, 128], f16, tag="tpb")
                    nc.tensor.matmul(tp[:], hid16[:], id16[:],
                                     is_transpose=True, start=True, stop=True)
                    hidT = pl.tile([H, 128], f16, tag="hidT")
                    nc.vector.tensor_copy(hidT[:], tp[:])
                    lg_ps = psh.tile([NC, 128], f32, tag="lg_ps")
                    nc.tensor.matmul(lg_ps[:], smaxw[:], hidT[:],
                                     start=True, stop=False)
                    nc.tensor.matmul(lg_ps[:], smaxb[:], ones16[:, 0:128],
                                     start=False, stop=True)
                    lgT = pl.tile([NC, 128], f32, tag="lgT")
                    nc.vector.tensor_copy(lgT[:], lg_ps[:])
                    plt = psh.tile([128, NC], f32, tag="plt")
                    nc.tensor.matmul(plt[:], lgT[:], id32[0:NC, 0:NC],
                                     is_transpose=True, start=True, stop=True)
                    lg = pl.tile([128, NC], f32, tag="lg")
                    nc.vector.tensor_copy(lg[:], plt[:])
                    m6 = pl.tile([128, 1], f32, tag="m6")
                    nc.vector.reduce_max(m6[:], lg[:], axis=AX.XYZW)
                    nm6 = pl.tile([128, 1], f32, tag="nm6")
                    nc.vector.tensor_scalar_mul(nm6[:], m6[:], -1.0)
                    e6 = pl.tile([128, NC], f32, tag="e6")
                    s6 = pl.tile([128, 1], f32, tag="s6")
                    nc.scalar.activation(e6[:], lg[:], AF.Exp, bias=nm6[:],
                                         scale=1.0, accum_out=s6[:])
                    ls6 = pl.tile([128, 1], f32, tag="ls6")
                    nc.scalar.activation(ls6[:], s6[:], AF.Ln)
                    sh = pl.tile([128, 1], f32, tag="sh")
                    nc.vector.tensor_add(sh[:], m6[:], ls6[:])
                    outt = pl.tile([128, NC], f32, tag="outt")
                    nc.vector.tensor_scalar(outt[:], lg[:], sh[:], None,
                                            op0=ALU.subtract)
                    nc.sync.dma_start(outd[qt * 128:(qt + 1) * 128, :],
                                      outt[:])
            pa.__exit__(None, None, None)
            pm.__exit__(None, None, None)

    nc.compile()
    return nc


# ------------------------------------------------------------------ entry
def kernel(x, edge_index, edge_norm, edge_type, basis, comp, root_w, root_b,
           gc_rel_w, gc_rel_b, gc_root_w, beta_w, beta_b, lin_w, lin_b,
           smax_w, smax_b):
    x = np.ascontiguousarray(np.asarray(x, np.float32))
    per_core, meta = _prep(x, edge_index, edge_type)

    import os
    phase = os.environ.get("KPHASE", "full")
    key = (phase, meta["totB1"], tuple(meta["B1"]))
    if key not in _ker_cache:
        _ker_cache[key] = _build(meta, phase)
    nc = _ker_cache[key]

    x16 = x.astype(np.float16)
    xT = np.ascontiguousarray(x.T)
    linw_pack = np.ascontiguousarray(
        np.asarray(lin_w, np.float16).reshape(3, 100, H).transpose(
            1, 0, 2)).reshape(100, 3 * H)
    bw = np.asarray(beta_w, np.float32)           # [300, 300]
    betawT = np.ascontiguousarray(
        bw.reshape(3, 100, 3, 100).transpose(1, 0, 2, 3)).reshape(100, -1)
    rootw32 = np.ascontiguousarray(
        np.asarray(root_w, np.float32).reshape(2, 100, H).transpose(
            1, 0, 2)).reshape(100, 2 * H)

    shared = dict(
        xT16=np.ascontiguousarray(x16.T),
        basis32=np.ascontiguousarray(
            np.asarray(basis, np.float32).reshape(NB, F * H)),
        compT32=np.ascontiguousarray(np.asarray(comp, np.float32).T),
        rootw32=rootw32,
        rootb32=np.asarray(root_b, np.float32).reshape(1, H),
        gcrel32=np.asarray(gc_rel_w, np.float32),
        gcrelb32=np.asarray(gc_rel_b, np.float32).reshape(1, H),
        gcroot32=np.asarray(gc_root_w, np.float32),
        betaw32=betawT,
        betab32=np.asarray(beta_b, np.float32).reshape(1, 3 * H),
        linw16=linw_pack,
        linb32=np.asarray(lin_b, np.float32).reshape(1, H),
        smaxw16=np.asarray(smax_w, np.float16),
        smaxb16=np.asarray(smax_b, np.float16).reshape(1, NC),
        ident16=np.eye(128, dtype=np.float16),
        ident32=np.eye(128, dtype=np.float32),
        ones32=np.ones((1, 512), np.float32),
        ones16=np.ones((1, NPC), np.float16),
    )
    in_maps = []
    for c in range(CORES):
        m = dict(shared)
        strip = xT[:, c * NPC:(c + 1) * NPC]
        m["xTs32"] = np.ascontiguousarray(
            strip.reshape(2, 100, NPC).transpose(1, 0, 2)).reshape(
            100, 2 * NPC)
        m.update(per_core[c])
        in_maps.append(m)

    res = run_bass_kernel_spmd(nc, in_maps, core_ids=list(range(CORES)),
                               trace_cores=[0])
    global _last_res
    _last_res = res
    if phase != "full":
        return [res.results[c]["dbg"] for c in range(CORES)]
    return np.concatenate([res.results[c]["out"] for c in range(CORES)],
                          axis=0)
